# revision 16
# baseline (speedup 1.0000x reference)
"""Causal attention on 8 TRN2 NeuronCores — two-phase, fp8-heavy version.

Phase 1 (NEFF-1): Q/K/V projections, all in fp8 DoubleRow matmuls (x and
weights fp8-quantized on host). K/V sharded over seq across cores; Q^T for
the core's own (strided) row blocks. Outputs: K^T, Q^T, V in fp8 (V in
kpos-block layout for the fp8 PV matmuls of phase 2) plus a bf16 copy of
V's first 128 rows (v0b) — softmax rows with few terms need better-than-
fp8 V. Compute order K -> Q -> V so the tail output DMA is small.

Phase 2 (NEFF-2): flash-style causal attention, Q rows sharded over cores
(strided 128-row blocks), K^T / V streamed chunk-wise from DRAM in fp8.
Scores AND PV via fp8 DoubleRow matmuls (2x rate). Softmax without max
subtraction (logits are bounded): exp runs on the scalar engine straight
from PSUM with per-chunk accumulated sums; P^T via tensor-engine
transposes (cheap: they pipeline at ~61ns). The first 128 kpos of tile 0
multiply in bf16 (P^T bf16 x v0b) to protect short softmax rows. Causal
masking only touches the two boundary chunks per tile via two additive
masks whose thresholds are tile-independent. Each tile's PV + epilogue +
output DMA is interleaved into the chunk stream as soon as its P^T tiles
exist, so the kernel tail is just the last tile's boundary chunk.

DMA: the tiny thr tensor and the critical first inputs (qt block 3, kt0)
issue on the scalar queue ahead of the bulk stream; kt/v interleave on the
sync queue in consumption order; outputs go out on the gpsimd queue.
"""

import numpy as np
import ml_dtypes
from contextlib import ExitStack

import concourse.bass as bass
import concourse.tile as tile
from concourse import bacc, mybir
from concourse.bass_utils import run_bass_kernel_spmd
from concourse.masks import make_identity

P = 128
SEQ = 4096
D = 1024
N_CORES = 8
RPC = SEQ // N_CORES          # 512
D_TILES = D // P              # 8
KCHUNK = 512
SEQ_CHUNKS = SEQ // KCHUNK    # 8
N_QTILES = RPC // P           # 4
TILE_CHUNKS = [2, 4, 6, 8]
SM_SCALE = 1.0 / 32.0
NEG_BIG = -1.0e9

BF16 = mybir.dt.bfloat16
F32 = mybir.dt.float32
F8 = mybir.dt.float8e4
NP_F8 = ml_dtypes.float8_e4m3
DR = mybir.MatmulPerfMode.DoubleRow

_CACHE = {}


# ---------------------------------------------------------------- NEFF 1
def _build_nc1():
    nc = bacc.Bacc("TRN2", target_bir_lowering=False, debug=False,
                   num_devices=N_CORES)
    # partition-dim-first layouts, contiguous per partition
    xc = nc.dram_tensor("xc", [P, D_TILES, KCHUNK], F8,
                        kind="ExternalInput").ap()
    xq = nc.dram_tensor("xq", [P, D_TILES, RPC], F8,
                        kind="ExternalInput").ap()
    wk = nc.dram_tensor("wk", [P, D_TILES, D_TILES, P], F8,
                        kind="ExternalInput").ap()
    wq = nc.dram_tensor("wq", [P, D_TILES, D_TILES, P], F8,
                        kind="ExternalInput").ap()
    wv = nc.dram_tensor("wv", [P, 2, D_TILES, KCHUNK], F8,
                        kind="ExternalInput").ap()
    # bf16 copies of x rows 0-127 and w_v: V's first 128 rows must be
    # better than fp8 (short softmax rows see V noise unaveraged)
    xb = nc.dram_tensor("xb", [P, D_TILES, P], BF16,
                        kind="ExternalInput").ap()
    wvb = nc.dram_tensor("wvb", [P, 2, D_TILES, KCHUNK], BF16,
                         kind="ExternalInput").ap()
    kt_o = nc.dram_tensor("kt", [P, D_TILES, KCHUNK], F8,
                          kind="ExternalOutput").ap()
    qt_o = nc.dram_tensor("qt", [N_QTILES, P, D_TILES, P], F8,
                          kind="ExternalOutput").ap()
    v_o = nc.dram_tensor("v", [P, 4, D], F8, kind="ExternalOutput").ap()
    v0b_o = nc.dram_tensor("v0b", [P, D], BF16, kind="ExternalOutput").ap()

    with tile.TileContext(nc) as tc, ExitStack() as ctx:
        wpool = ctx.enter_context(tc.tile_pool(name="w", bufs=1))
        xpool = ctx.enter_context(tc.tile_pool(name="x", bufs=1))
        opool = ctx.enter_context(tc.tile_pool(name="o", bufs=1))
        ps = ctx.enter_context(tc.tile_pool(name="ps", bufs=6, space="PSUM"))

        # inputs stream on sync in consumption order (K -> Q -> V),
        # split so the first matmuls can start early
        xs = xpool.tile([P, D_TILES, KCHUNK], F8, tag="xs")
        nc.sync.dma_start(out=xs, in_=xc)
        wk_sb = wpool.tile([P, D_TILES, D_TILES, P], F8, tag="wk")
        nc.sync.dma_start(out=wk_sb[:, 0:1], in_=wk[:, 0:1])
        nc.sync.dma_start(out=wk_sb[:, 1:4], in_=wk[:, 1:4])
        nc.sync.dma_start(out=wk_sb[:, 4:8], in_=wk[:, 4:8])
        xq_sb = xpool.tile([P, D_TILES, RPC], F8, tag="xq")
        nc.sync.dma_start(out=xq_sb, in_=xq)
        wq_sb = wpool.tile([P, D_TILES, D_TILES, P], F8, tag="wq")
        nc.sync.dma_start(out=wq_sb[:, 0:4], in_=wq[:, 0:4])
        nc.sync.dma_start(out=wq_sb[:, 4:8], in_=wq[:, 4:8])
        wv_sb = wpool.tile([P, 2, D_TILES, KCHUNK], F8, tag="wv")
        nc.sync.dma_start(out=wv_sb, in_=wv)
        xb_sb = xpool.tile([P, D_TILES, P], BF16, tag="xb")
        nc.sync.dma_start(out=xb_sb, in_=xb)
        wvb_sb = wpool.tile([P, 2, D_TILES, KCHUNK], BF16, tag="wvb")
        nc.sync.dma_start(out=wvb_sb, in_=wvb)

        # HAM warmup: the PE sits idle >3.4us during the input DMA head,
        # so the first real matmuls would run at the cold 1.2 GHz clock.
        # ~16 dummy matmuls on a memset tile (ready ~6us, long before the
        # inputs land) trip the activity monitor to full rate.
        wz = xpool.tile([P, KCHUNK], BF16, tag="warm")
        nc.gpsimd.memset(wz, 0.0)
        wps = ps.tile([P, KCHUNK], F32, tag="ps")
        for _ in range(16):
            nc.tensor.matmul(wps, wz[:, 0:P], wz, start=True, stop=True)

        n_copy = 0

        def evac(dst, src):
            nonlocal n_copy
            if n_copy % 2 == 0:
                nc.vector.tensor_copy(dst, src)
            else:
                nc.scalar.copy(dst, src)
            n_copy += 1

        # K^T: [d_out, seq-chunk] fp8, one output DMA
        kt_sb = opool.tile([P, D_TILES, KCHUNK], F8, tag="kt")
        for do in range(D_TILES):
            p = ps.tile([P, KCHUNK], F32, tag="ps")
            for dp in range(D_TILES // 2):
                nc.tensor.matmul(p, wk_sb[:, do, 2 * dp:2 * dp + 2, :],
                                 xs[:, 2 * dp:2 * dp + 2, :],
                                 start=(dp == 0), stop=(dp == 3),
                                 perf_mode=DR)
            evac(kt_sb[:, do, :], p)
        nc.gpsimd.dma_start(out=kt_o, in_=kt_sb)

        # Q^T: fp8, block-major output (phase 2 fetches block 3 first)
        qt_sb = opool.tile([P, D_TILES, RPC], F8, tag="qt")
        for do in range(D_TILES):
            p = ps.tile([P, RPC], F32, tag="ps")
            for dp in range(D_TILES // 2):
                nc.tensor.matmul(p, wq_sb[:, do, 2 * dp:2 * dp + 2, :],
                                 xq_sb[:, 2 * dp:2 * dp + 2, :],
                                 start=(dp == 0), stop=(dp == 3),
                                 perf_mode=DR)
            evac(qt_sb[:, do, :], p)
        for b in range(N_QTILES):
            nc.gpsimd.dma_start(out=qt_o[b],
                                in_=qt_sb[:, :, b * P:(b + 1) * P])

        # V: [kpos-block, d_out] fp8 + bf16 copy of the first 128 rows
        v_sb = opool.tile([P, 4, D], F8, tag="v")
        v0b_sb = opool.tile([P, D], BF16, tag="v0b")
        for ks in range(4):
            for h in range(2):
                p = ps.tile([P, KCHUNK], F32, tag="ps")
                for dp in range(D_TILES // 2):
                    nc.tensor.matmul(p, xs[:, 2 * dp:2 * dp + 2,
                                           ks * P:(ks + 1) * P],
                                     wv_sb[:, h, 2 * dp:2 * dp + 2, :],
                                     start=(dp == 0), stop=(dp == 3),
                                     perf_mode=DR)
                evac(v_sb[:, ks, h * KCHUNK:(h + 1) * KCHUNK], p)
        nc.gpsimd.dma_start(out=v_o, in_=v_sb)
        # V rows 0-127 again, in bf16 from bf16 inputs
        for h in range(2):
            p = ps.tile([P, KCHUNK], F32, tag="ps")
            for di in range(D_TILES):
                nc.tensor.matmul(p, xb_sb[:, di, :], wvb_sb[:, h, di, :],
                                 start=(di == 0), stop=(di == D_TILES - 1))
            nc.scalar.copy(v0b_sb[:, h * KCHUNK:(h + 1) * KCHUNK], p)
        nc.gpsimd.dma_start(out=v0b_o, in_=v0b_sb)
    nc.compile()
    return nc


# ---------------------------------------------------------------- NEFF 2
def _build_nc2():
    nc = bacc.Bacc("TRN2", target_bir_lowering=False, debug=False,
                   num_devices=N_CORES)
    ktf = nc.dram_tensor("ktf", [SEQ_CHUNKS, P, D_TILES, KCHUNK], F8,
                         kind="ExternalInput").ap()
    vf = nc.dram_tensor("vf", [SEQ_CHUNKS, P, 4, D], F8,
                        kind="ExternalInput").ap()
    qt = nc.dram_tensor("qt", [N_QTILES, P, D_TILES, P], F8,
                        kind="ExternalInput").ap()
    vb0 = nc.dram_tensor("vb0", [P, D], BF16, kind="ExternalInput").ap()
    thr = nc.dram_tensor("thr", [P, 2], F32, kind="ExternalInput").ap()
    out = nc.dram_tensor("out", [RPC, D], F32, kind="ExternalOutput").ap()
    out_t = out.rearrange("(t p) f -> p t f", p=P)

    with tile.TileContext(nc) as tc, ExitStack() as ctx:
        _attention(ctx, tc, ktf, vf, qt, vb0, thr, out_t)
    nc.compile()
    return nc


def _attention(ctx, tc, ktf, vf, qt_in, vb0_in, thr_in, out_t):
    nc = tc.nc
    AX = mybir.AxisListType
    OP = mybir.AluOpType
    ACT = mybir.ActivationFunctionType

    consts = ctx.enter_context(tc.tile_pool(name="consts", bufs=1))
    qt_pool = ctx.enter_context(tc.tile_pool(name="qt", bufs=1))
    kt_pool = ctx.enter_context(tc.tile_pool(name="kt", bufs=1))
    vres_pool = ctx.enter_context(tc.tile_pool(name="vres", bufs=1))
    pt_pool = ctx.enter_context(tc.tile_pool(name="pt", bufs=1))
    p_pool = ctx.enter_context(tc.tile_pool(name="p", bufs=9))
    stmp_pool = ctx.enter_context(tc.tile_pool(name="stmp", bufs=4))
    osb_pool = ctx.enter_context(tc.tile_pool(name="osb", bufs=2))
    stat_pool = ctx.enter_context(tc.tile_pool(name="stat", bufs=16))

    s_ps = ctx.enter_context(tc.tile_pool(name="s_ps", bufs=2, space="PSUM"))
    t_ps = ctx.enter_context(tc.tile_pool(name="t_ps", bufs=2, space="PSUM"))
    o_ps = ctx.enter_context(tc.tile_pool(name="o_ps", bufs=2, space="PSUM"))

    # ALL inputs on one HWDGE queue in strict priority order — a second
    # queue's descriptors would round-robin into the same 16 rings and
    # halve the critical stream's bandwidth. thr (1 KB, gates the masks)
    # rides the otherwise-idle scalar queue so it lands instantly.
    thr_sb = consts.tile([P, 2], F32)
    nc.scalar.dma_start(out=thr_sb, in_=thr_in)
    qt_sb = qt_pool.tile([P, D_TILES, RPC], F8)
    kts = [kt_pool.tile([P, D_TILES, KCHUNK], F8, tag=f"kt{j}",
                        name=f"kt{j}")
           for j in range(SEQ_CHUNKS)]
    v_res = [vres_pool.tile([P, 4, D], F8, tag=f"v{j}", name=f"vres{j}")
             for j in range(SEQ_CHUNKS)]
    nc.sync.dma_start(out=qt_sb[:, :, 3 * P:4 * P], in_=qt_in[3])
    nc.sync.dma_start(out=kts[0][:, 0:4], in_=ktf[0][:, 0:4])
    nc.sync.dma_start(out=kts[0][:, 4:8], in_=ktf[0][:, 4:8])
    for b in (2, 1, 0):
        nc.sync.dma_start(out=qt_sb[:, :, b * P:(b + 1) * P], in_=qt_in[b])
    vb0_sb = consts.tile([P, D], BF16)
    # bulk stream in consumption order (S eats kt_j at round j, PV(t)
    # eats v_0..v_{2t+1} at round 2t+2)
    nc.sync.dma_start(out=kts[1], in_=ktf[1])
    nc.sync.dma_start(out=vb0_sb, in_=vb0_in)
    nc.sync.dma_start(out=v_res[0], in_=vf[0])
    nc.sync.dma_start(out=v_res[1], in_=vf[1])
    for j in range(2, SEQ_CHUNKS):
        nc.sync.dma_start(out=kts[j], in_=ktf[j])
        nc.sync.dma_start(out=v_res[j], in_=vf[j])

    # HAM warmup during the input-DMA head (see phase 1)
    wz = consts.tile([P, KCHUNK], BF16, name="warm")
    nc.gpsimd.memset(wz, 0.0)
    wps = s_ps.tile([P, KCHUNK], F32, tag="s")
    for _ in range(16):
        nc.tensor.matmul(wps, wz[:, 0:P], wz, start=True, stop=True)

    ident = consts.tile([P, P], BF16)
    make_identity(nc, ident)
    iota_i = consts.tile([P, KCHUNK], mybir.dt.int32)
    nc.gpsimd.iota(iota_i, pattern=[[1, KCHUNK]], base=0, channel_multiplier=0)
    iota_f = consts.tile([P, KCHUNK], F32)
    nc.vector.tensor_copy(iota_f, iota_i)
    negbig = consts.tile([P, KCHUNK], F32)
    nc.gpsimd.memset(negbig, NEG_BIG)
    # additive causal masks for the two boundary chunks of every tile:
    # col i of chunk 2t+m is masked for row r iff i >= thr[r, m]
    mk = consts.tile([P, 2, KCHUNK], BF16)
    for m in range(2):
        nc.vector.scalar_tensor_tensor(mk[:, m, :], iota_f,
                                       thr_sb[:, m:m + 1], negbig,
                                       op0=OP.is_ge, op1=OP.mult)

    # per-tile P^T (fp8, [kpos-block, q] layout) and chunk-sum slots
    ptjs = [pt_pool.tile([P, TILE_CHUNKS[t], 4, P], F8, tag=f"pt{t}",
                         name=f"ptj{t}")
            for t in range(N_QTILES)]
    ptb0 = consts.tile([P, P], BF16, name="ptb0")
    partials = [stat_pool.tile([P, SEQ_CHUNKS], F32, tag=f"pa{t}",
                               name=f"partials{t}")
                for t in range(N_QTILES)]

    p_exps = {}
    n_tcopy = 0
    o_tiles = {}

    def emit_s(t, j):
        """S chunk (fp8 DoubleRow) -> exp (scalar, accumulated sums).
        Non-boundary chunks exp straight from PSUM; boundary chunks fold
        the additive mask in a vector add first."""
        ktj = kts[j]
        sps = s_ps.tile([P, KCHUNK], F32, tag="s")
        for dp in range(D_TILES // 2):
            nc.tensor.matmul(sps,
                             qt_sb[:, 2 * dp:2 * dp + 2, t * P:(t + 1) * P],
                             ktj[:, 2 * dp:2 * dp + 2, :],
                             start=(dp == 0), stop=(dp == D_TILES // 2 - 1),
                             perf_mode=DR)
        p_sb = p_pool.tile([P, KCHUNK], BF16, tag="p")
        if j >= 2 * t:
            srt = stmp_pool.tile([P, KCHUNK], BF16, tag="stmp")
            nc.vector.tensor_tensor(srt, sps, mk[:, j - 2 * t, :], OP.add)
            nc.scalar.activation(p_sb, srt, ACT.Exp, scale=SM_SCALE,
                                 accum_out=partials[t][:, j:j + 1])
        else:
            nc.scalar.activation(p_sb, sps, ACT.Exp, scale=SM_SCALE,
                                 accum_out=partials[t][:, j:j + 1])
        p_exps[(t, j)] = p_sb

    def emit_transposes(t, j):
        nonlocal n_tcopy
        tps = t_ps.tile([P, KCHUNK], BF16)
        p_sb = p_exps.pop((t, j))
        for ks in range(4):
            nc.tensor.transpose(tps[:, ks * P:(ks + 1) * P],
                                p_sb[:, ks * P:(ks + 1) * P], ident)
        if t == 0 and j == 0:
            # kpos 0-127 keep a bf16 P^T (bf16 PV protects short rows)
            nc.vector.tensor_copy(ptb0, tps[:, 0:P])
            nc.scalar.copy(ptjs[0][:, 0, 1:4, :], tps[:, P:])
        elif n_tcopy % 2 == 0:
            nc.vector.tensor_copy(ptjs[t][:, j], tps)
        else:
            nc.scalar.copy(ptjs[t][:, j], tps)
        n_tcopy += 1

    def emit_pv(t, kcs, ops, stop):
        """PV chunks (fp8 DoubleRow; bf16 for tile 0's first kpos block)."""
        for kc in kcs:
            if t == 0 and kc == 0:
                for h in range(2):
                    nc.tensor.matmul(ops[:, h * KCHUNK:(h + 1) * KCHUNK],
                                     ptb0,
                                     vb0_sb[:, h * KCHUNK:(h + 1) * KCHUNK],
                                     start=True, stop=False)
                for h in range(2):
                    nc.tensor.matmul(ops[:, h * KCHUNK:(h + 1) * KCHUNK],
                                     ptjs[0][:, 0, 1:3, :],
                                     v_res[0][:, 1:3,
                                              h * KCHUNK:(h + 1) * KCHUNK],
                                     start=False, stop=False, perf_mode=DR)
                for h in range(2):
                    nc.tensor.matmul(ops[:, h * KCHUNK:(h + 1) * KCHUNK],
                                     ptjs[0][:, 0, 3, :],
                                     v_res[0][:, 3,
                                              h * KCHUNK:(h + 1) * KCHUNK],
                                     start=False, stop=False)
                continue
            for m in range(2):
                last = stop and kc == kcs[-1] and m == 1
                for h in range(2):
                    nc.tensor.matmul(ops[:, h * KCHUNK:(h + 1) * KCHUNK],
                                     ptjs[t][:, kc, 2 * m:2 * m + 2, :],
                                     v_res[kc][:, 2 * m:2 * m + 2,
                                               h * KCHUNK:(h + 1) * KCHUNK],
                                     start=(kc == 0 and m == 0),
                                     stop=last, perf_mode=DR)

    def emit_epilogue(t, ops):
        n = TILE_CHUNKS[t]
        rsum = stat_pool.tile([P, 1], F32, tag="stat")
        nc.vector.tensor_reduce(rsum, partials[t][:, :n], op=OP.add,
                                axis=AX.X)
        recip = stat_pool.tile([P, 1], F32, tag="stat")
        nc.vector.reciprocal(recip, rsum)
        # per-bank mul + DMA: the h0 half starts as soon as its PSUM bank
        # gets its stop-flag, overlapping the h1 half's last PV matmuls
        osb = osb_pool.tile([P, D], F32)
        for h in range(2):
            nc.vector.tensor_scalar_mul(osb[:, h * KCHUNK:(h + 1) * KCHUNK],
                                        ops[:, h * KCHUNK:(h + 1) * KCHUNK],
                                        recip)
            nc.gpsimd.dma_start(out=out_t[:, t, h * KCHUNK:(h + 1) * KCHUNK],
                                in_=osb[:, h * KCHUNK:(h + 1) * KCHUNK])

    # ---- fused stream ---------------------------------------------------
    for j in range(SEQ_CHUNKS + 1):
        # t descending: the deepest tile is on the critical path
        for t in reversed(range(N_QTILES)):
            if j < TILE_CHUNKS[t] and not (j == 7 and t == 3):
                emit_s(t, j)
        for t in reversed(range(N_QTILES)):
            if 1 <= j <= TILE_CHUNKS[t]:
                emit_transposes(t, j - 1)
        # PV as soon as a tile's P^T blocks are all (or nearly all) in:
        # short tiles complete right after their last transpose; tile 3
        # runs chunks 0-6 during round 7 (ahead of S(3,7), so a late kt7
        # can't stall the in-order tensor queue) and chunk 7 at round 8
        if j == 2 or j == 4 or j == 6:
            t = j // 2 - 1
            ops = o_ps.tile([P, D], F32)
            emit_pv(t, range(TILE_CHUNKS[t]), ops, stop=True)
            emit_epilogue(t, ops)
        elif j == 7:
            ops = o_ps.tile([P, D], F32)
            o_tiles[3] = ops
            emit_pv(3, range(7), ops, stop=False)
            emit_s(3, 7)
        elif j == 8:
            ops = o_tiles[3]
            emit_pv(3, [7], ops, stop=True)
            emit_epilogue(3, ops)


def _get_ncs():
    if "nc1" not in _CACHE:
        _CACHE["nc1"] = _build_nc1()
        _CACHE["nc2"] = _build_nc2()
    return _CACHE["nc1"], _CACHE["nc2"]


def _qcols(c):
    blocks = [8 * t + c for t in range(N_QTILES)]
    return blocks, np.concatenate(
        [np.arange(b * P, (b + 1) * P) for b in blocks])


def _perm_x(xT_slice):
    """[D, W] -> [128, 8, W] fp8 with di_inner on partitions."""
    W = xT_slice.shape[1]
    return np.ascontiguousarray(
        xT_slice.reshape(D_TILES, P, W).transpose(1, 0, 2)).astype(NP_F8)


def _perm_w_chunks(wT):
    """[d_in, d_out] -> [128, 8, 8, 128]: [di_p, do_o, di_o, do_i] fp8."""
    return np.ascontiguousarray(
        np.asarray(wT, np.float32).reshape(D_TILES, P, D_TILES, P)
        .transpose(1, 2, 0, 3)).astype(NP_F8)


def _perm_w_halves(wT):
    """[d_in, d_out] -> [128, 2, 8, 512]: [di_p, half, di_o, do_i] fp8."""
    return np.ascontiguousarray(
        np.asarray(wT, np.float32).reshape(D_TILES, P, 2, KCHUNK)
        .transpose(1, 2, 0, 3)).astype(NP_F8)


def _phase1_inmaps(xT, wqT, wkT, wvT):
    bf = ml_dtypes.bfloat16
    wk_p = _perm_w_chunks(wkT)
    wq_p = _perm_w_chunks(wqT)
    wv_p = _perm_w_halves(wvT)
    wvb_p = np.ascontiguousarray(
        np.asarray(wvT, np.float32).reshape(D_TILES, P, 2, KCHUNK)
        .transpose(1, 2, 0, 3)).astype(bf)
    xb_p = np.ascontiguousarray(
        np.asarray(xT[:, 0:P], np.float32)
        .reshape(D_TILES, P, P).transpose(1, 0, 2)).astype(bf)
    maps = []
    for c in range(N_CORES):
        _, cols = _qcols(c)
        maps.append({
            "xc": _perm_x(xT[:, c * KCHUNK:(c + 1) * KCHUNK]),
            "xq": _perm_x(xT[:, cols]),
            "wq": wq_p, "wk": wk_p, "wv": wv_p,
            "xb": xb_p, "wvb": wvb_p})
    return maps


def _gather_phase1(res1):
    ktf = np.stack([res1.results[c]["kt"] for c in range(N_CORES)])
    vf = np.stack([res1.results[c]["v"] for c in range(N_CORES)])
    qts = [res1.results[c]["qt"] for c in range(N_CORES)]
    vb0 = res1.results[0]["v0b"]
    return ktf, vf, qts, vb0


def _phase2_inmaps(ktf, vf, qts, vb0):
    maps = []
    r = np.arange(P, dtype=np.float32)
    for c in range(N_CORES):
        thr = np.empty((P, 2), np.float32)
        thr[:, 0] = 128 * c + r + 1
        thr[:, 1] = 128 * c + r + 1 - KCHUNK
        maps.append({"ktf": ktf, "vf": vf, "qt": qts[c], "vb0": vb0,
                     "thr": thr})
    return maps


def _run_spmd(nc, in_maps):
    """run_bass_kernel_spmd with retries: the first device touch after a
    crashed process occasionally reports NRT_EXEC_UNIT_UNRECOVERABLE once."""
    last = None
    for _ in range(3):
        try:
            return run_bass_kernel_spmd(nc, in_maps, list(range(N_CORES)))
        except Exception as e:  # transient device wedge
            last = e
    raise last


def kernel(x, w_q, w_k, w_v):
    nc1, nc2 = _get_ncs()
    x = np.asarray(x, np.float32)
    xT = np.ascontiguousarray(x.T)
    wqT = np.ascontiguousarray(np.asarray(w_q).T)
    wkT = np.ascontiguousarray(np.asarray(w_k).T)
    wvT = np.ascontiguousarray(np.asarray(w_v).T)

    res1 = _run_spmd(nc1, _phase1_inmaps(xT, wqT, wkT, wvT))
    res2 = _run_spmd(nc2, _phase2_inmaps(*_gather_phase1(res1)))

    full = np.empty((SEQ, D), np.float32)
    for c in range(N_CORES):
        oc = res2.results[c]["out"]
        blocks, _ = _qcols(c)
        for t, B in enumerate(blocks):
            full[B * P:(B + 1) * P, :] = oc[t * P:(t + 1) * P, :]
    return full


# revision 17
# speedup vs baseline: 1.1606x; 1.1606x over previous
"""Causal attention on 8 TRN2 NeuronCores — two-phase, fp8-heavy version.

Phase 1 (NEFF-1): Q/K/V projections, all in fp8 DoubleRow matmuls (x and
weights fp8-quantized on host). K/V sharded over seq across cores; Q^T for
the core's own (strided) row blocks. Outputs: K^T, Q^T, V in fp8 (V in
kpos-block layout for the fp8 PV matmuls of phase 2) plus a bf16 copy of
V's first 128 rows (v0b) — softmax rows with few terms need better-than-
fp8 V. Compute order K -> Q -> V so the tail output DMA is small.

Phase 2 (NEFF-2): flash-style causal attention, Q rows sharded over cores
(strided 128-row blocks), K^T / V streamed chunk-wise from DRAM in fp8.
Scores AND PV via fp8 DoubleRow matmuls (2x rate). Softmax without max
subtraction (logits are bounded): exp runs on the scalar engine straight
from PSUM with per-chunk accumulated sums; P^T via tensor-engine
transposes (cheap: they pipeline at ~61ns). The first 128 kpos of tile 0
multiply in bf16 (P^T bf16 x v0b) to protect short softmax rows. Causal
masking only touches the two boundary chunks per tile via two additive
masks whose thresholds are tile-independent. Each tile's PV + epilogue +
output DMA is interleaved into the chunk stream as soon as its P^T tiles
exist, so the kernel tail is just the last tile's boundary chunk.

DMA: the tiny thr tensor and the critical first inputs (qt block 3, kt0)
issue on the scalar queue ahead of the bulk stream; kt/v interleave on the
sync queue in consumption order; outputs go out on the gpsimd queue.
"""

import numpy as np
import ml_dtypes
from contextlib import ExitStack

import concourse.bass as bass
import concourse.tile as tile
from concourse import bacc, mybir
from concourse.bass_utils import run_bass_kernel_spmd
from concourse.masks import make_identity

P = 128
SEQ = 4096
D = 1024
N_CORES = 8
RPC = SEQ // N_CORES          # 512
D_TILES = D // P              # 8
KCHUNK = 512
SEQ_CHUNKS = SEQ // KCHUNK    # 8
N_QTILES = RPC // P           # 4
TILE_CHUNKS = [2, 4, 6, 8]
SM_SCALE = 1.0 / 32.0
NEG_BIG = -1.0e9

BF16 = mybir.dt.bfloat16
F32 = mybir.dt.float32
F8 = mybir.dt.float8e4
NP_F8 = ml_dtypes.float8_e4m3
DR = mybir.MatmulPerfMode.DoubleRow

_CACHE = {}


# ---------------------------------------------------------------- NEFF 1
def _build_nc1():
    nc = bacc.Bacc("TRN2", target_bir_lowering=False, debug=False,
                   num_devices=N_CORES)
    # partition-dim-first layouts, contiguous per partition
    xc = nc.dram_tensor("xc", [P, D_TILES, KCHUNK], F8,
                        kind="ExternalInput").ap()
    xq = nc.dram_tensor("xq", [P, D_TILES, RPC], F8,
                        kind="ExternalInput").ap()
    wk = nc.dram_tensor("wk", [P, D_TILES, D_TILES, P], F8,
                        kind="ExternalInput").ap()
    wq = nc.dram_tensor("wq", [P, D_TILES, D_TILES, P], F8,
                        kind="ExternalInput").ap()
    wv = nc.dram_tensor("wv", [P, 2, D_TILES, KCHUNK], F8,
                        kind="ExternalInput").ap()
    # bf16 copies of x rows 0-127 and w_v: V's first 128 rows must be
    # better than fp8 (short softmax rows see V noise unaveraged)
    xb = nc.dram_tensor("xb", [P, D_TILES, P], BF16,
                        kind="ExternalInput").ap()
    wvb = nc.dram_tensor("wvb", [P, 2, D_TILES, KCHUNK], BF16,
                         kind="ExternalInput").ap()
    kt_o = nc.dram_tensor("kt", [P, D_TILES, KCHUNK], F8,
                          kind="ExternalOutput").ap()
    qt_o = nc.dram_tensor("qt", [N_QTILES, P, D_TILES, P], F8,
                          kind="ExternalOutput").ap()
    v_o = nc.dram_tensor("v", [P, 4, D], F8, kind="ExternalOutput").ap()
    v0b_o = nc.dram_tensor("v0b", [P, D], BF16, kind="ExternalOutput").ap()

    with tile.TileContext(nc) as tc, ExitStack() as ctx:
        wpool = ctx.enter_context(tc.tile_pool(name="w", bufs=1))
        xpool = ctx.enter_context(tc.tile_pool(name="x", bufs=1))
        opool = ctx.enter_context(tc.tile_pool(name="o", bufs=1))
        ps = ctx.enter_context(tc.tile_pool(name="ps", bufs=6, space="PSUM"))

        # inputs stream on sync in consumption order (K -> Q -> V),
        # split so the first matmuls can start early
        xs = xpool.tile([P, D_TILES, KCHUNK], F8, tag="xs")
        nc.sync.dma_start(out=xs, in_=xc)
        wk_sb = wpool.tile([P, D_TILES, D_TILES, P], F8, tag="wk")
        nc.sync.dma_start(out=wk_sb[:, 0:1], in_=wk[:, 0:1])
        nc.sync.dma_start(out=wk_sb[:, 1:4], in_=wk[:, 1:4])
        nc.sync.dma_start(out=wk_sb[:, 4:8], in_=wk[:, 4:8])
        xq_sb = xpool.tile([P, D_TILES, RPC], F8, tag="xq")
        nc.sync.dma_start(out=xq_sb, in_=xq)
        wq_sb = wpool.tile([P, D_TILES, D_TILES, P], F8, tag="wq")
        nc.sync.dma_start(out=wq_sb[:, 0:4], in_=wq[:, 0:4])
        nc.sync.dma_start(out=wq_sb[:, 4:8], in_=wq[:, 4:8])
        wv_sb = wpool.tile([P, 2, D_TILES, KCHUNK], F8, tag="wv")
        nc.sync.dma_start(out=wv_sb, in_=wv)
        xb_sb = xpool.tile([P, D_TILES, P], BF16, tag="xb")
        nc.sync.dma_start(out=xb_sb, in_=xb)
        wvb_sb = wpool.tile([P, 2, D_TILES, KCHUNK], BF16, tag="wvb")
        nc.sync.dma_start(out=wvb_sb, in_=wvb)

        n_copy = 0

        def evac(dst, src):
            nonlocal n_copy
            if n_copy % 2 == 0:
                nc.vector.tensor_copy(dst, src)
            else:
                nc.scalar.copy(dst, src)
            n_copy += 1

        # K^T: [d_out, seq-chunk] fp8, one output DMA
        kt_sb = opool.tile([P, D_TILES, KCHUNK], F8, tag="kt")
        for do in range(D_TILES):
            p = ps.tile([P, KCHUNK], F32, tag="ps")
            for dp in range(D_TILES // 2):
                nc.tensor.matmul(p, wk_sb[:, do, 2 * dp:2 * dp + 2, :],
                                 xs[:, 2 * dp:2 * dp + 2, :],
                                 start=(dp == 0), stop=(dp == 3),
                                 perf_mode=DR)
            evac(kt_sb[:, do, :], p)
        nc.gpsimd.dma_start(out=kt_o, in_=kt_sb)

        # Q^T: fp8, block-major output (phase 2 fetches block 3 first)
        qt_sb = opool.tile([P, D_TILES, RPC], F8, tag="qt")
        for do in range(D_TILES):
            p = ps.tile([P, RPC], F32, tag="ps")
            for dp in range(D_TILES // 2):
                nc.tensor.matmul(p, wq_sb[:, do, 2 * dp:2 * dp + 2, :],
                                 xq_sb[:, 2 * dp:2 * dp + 2, :],
                                 start=(dp == 0), stop=(dp == 3),
                                 perf_mode=DR)
            evac(qt_sb[:, do, :], p)
        for b in range(N_QTILES):
            nc.gpsimd.dma_start(out=qt_o[b],
                                in_=qt_sb[:, :, b * P:(b + 1) * P])

        # V: [kpos-block, d_out] fp8 + bf16 copy of the first 128 rows
        v_sb = opool.tile([P, 4, D], F8, tag="v")
        v0b_sb = opool.tile([P, D], BF16, tag="v0b")
        for ks in range(4):
            for h in range(2):
                p = ps.tile([P, KCHUNK], F32, tag="ps")
                for dp in range(D_TILES // 2):
                    nc.tensor.matmul(p, xs[:, 2 * dp:2 * dp + 2,
                                           ks * P:(ks + 1) * P],
                                     wv_sb[:, h, 2 * dp:2 * dp + 2, :],
                                     start=(dp == 0), stop=(dp == 3),
                                     perf_mode=DR)
                evac(v_sb[:, ks, h * KCHUNK:(h + 1) * KCHUNK], p)
        nc.gpsimd.dma_start(out=v_o, in_=v_sb)
        # V rows 0-127 again, in bf16 from bf16 inputs
        for h in range(2):
            p = ps.tile([P, KCHUNK], F32, tag="ps")
            for di in range(D_TILES):
                nc.tensor.matmul(p, xb_sb[:, di, :], wvb_sb[:, h, di, :],
                                 start=(di == 0), stop=(di == D_TILES - 1))
            nc.scalar.copy(v0b_sb[:, h * KCHUNK:(h + 1) * KCHUNK], p)
        nc.gpsimd.dma_start(out=v0b_o, in_=v0b_sb)
    nc.compile()
    return nc


# ---------------------------------------------------------------- NEFF 2
def _build_nc2():
    nc = bacc.Bacc("TRN2", target_bir_lowering=False, debug=False,
                   num_devices=N_CORES)
    ktf = nc.dram_tensor("ktf", [SEQ_CHUNKS, P, D_TILES, KCHUNK], F8,
                         kind="ExternalInput").ap()
    vf = nc.dram_tensor("vf", [SEQ_CHUNKS, P, 4, D], F8,
                        kind="ExternalInput").ap()
    qt = nc.dram_tensor("qt", [N_QTILES, P, D_TILES, P], F8,
                        kind="ExternalInput").ap()
    vb0 = nc.dram_tensor("vb0", [P, D], BF16, kind="ExternalInput").ap()
    thr = nc.dram_tensor("thr", [P, 2], F32, kind="ExternalInput").ap()
    out = nc.dram_tensor("out", [RPC, D], F32, kind="ExternalOutput").ap()
    out_t = out.rearrange("(t p) f -> p t f", p=P)

    with tile.TileContext(nc) as tc, ExitStack() as ctx:
        _attention(ctx, tc, ktf, vf, qt, vb0, thr, out_t)
    nc.compile()
    return nc


def _attention(ctx, tc, ktf, vf, qt_in, vb0_in, thr_in, out_t):
    nc = tc.nc
    AX = mybir.AxisListType
    OP = mybir.AluOpType
    ACT = mybir.ActivationFunctionType

    consts = ctx.enter_context(tc.tile_pool(name="consts", bufs=1))
    qt_pool = ctx.enter_context(tc.tile_pool(name="qt", bufs=1))
    kt_pool = ctx.enter_context(tc.tile_pool(name="kt", bufs=1))
    vres_pool = ctx.enter_context(tc.tile_pool(name="vres", bufs=1))
    pt_pool = ctx.enter_context(tc.tile_pool(name="pt", bufs=1))
    p_pool = ctx.enter_context(tc.tile_pool(name="p", bufs=9))
    stmp_pool = ctx.enter_context(tc.tile_pool(name="stmp", bufs=4))
    osb_pool = ctx.enter_context(tc.tile_pool(name="osb", bufs=2))
    stat_pool = ctx.enter_context(tc.tile_pool(name="stat", bufs=16))

    s_ps = ctx.enter_context(tc.tile_pool(name="s_ps", bufs=2, space="PSUM"))
    t_ps = ctx.enter_context(tc.tile_pool(name="t_ps", bufs=2, space="PSUM"))
    o_ps = ctx.enter_context(tc.tile_pool(name="o_ps", bufs=2, space="PSUM"))

    # ALL inputs on one HWDGE queue in strict priority order — a second
    # queue's descriptors would round-robin into the same 16 rings and
    # halve the critical stream's bandwidth. thr (1 KB, gates the masks)
    # rides the otherwise-idle scalar queue so it lands instantly.
    thr_sb = consts.tile([P, 2], F32)
    nc.scalar.dma_start(out=thr_sb, in_=thr_in)
    qt_sb = qt_pool.tile([P, D_TILES, RPC], F8)
    kts = [kt_pool.tile([P, D_TILES, KCHUNK], F8, tag=f"kt{j}",
                        name=f"kt{j}")
           for j in range(SEQ_CHUNKS)]
    v_res = [vres_pool.tile([P, 4, D], F8, tag=f"v{j}", name=f"vres{j}")
             for j in range(SEQ_CHUNKS)]
    nc.sync.dma_start(out=qt_sb[:, :, 3 * P:4 * P], in_=qt_in[3])
    nc.sync.dma_start(out=kts[0][:, 0:4], in_=ktf[0][:, 0:4])
    nc.sync.dma_start(out=kts[0][:, 4:8], in_=ktf[0][:, 4:8])
    for b in (2, 1, 0):
        nc.sync.dma_start(out=qt_sb[:, :, b * P:(b + 1) * P], in_=qt_in[b])
    vb0_sb = consts.tile([P, D], BF16)
    # bulk stream in consumption order (S eats kt_j at round j, PV(t)
    # eats v_0..v_{2t+1} at round 2t+2)
    nc.sync.dma_start(out=kts[1], in_=ktf[1])
    nc.sync.dma_start(out=vb0_sb, in_=vb0_in)
    nc.sync.dma_start(out=v_res[0], in_=vf[0])
    nc.sync.dma_start(out=v_res[1], in_=vf[1])
    for j in range(2, SEQ_CHUNKS):
        nc.sync.dma_start(out=kts[j], in_=ktf[j])
        nc.sync.dma_start(out=v_res[j], in_=vf[j])

    ident = consts.tile([P, P], BF16)
    make_identity(nc, ident)
    iota_i = consts.tile([P, KCHUNK], mybir.dt.int32)
    nc.gpsimd.iota(iota_i, pattern=[[1, KCHUNK]], base=0, channel_multiplier=0)
    iota_f = consts.tile([P, KCHUNK], F32)
    nc.vector.tensor_copy(iota_f, iota_i)
    negbig = consts.tile([P, KCHUNK], F32)
    nc.gpsimd.memset(negbig, NEG_BIG)
    # additive causal masks for the two boundary chunks of every tile:
    # col i of chunk 2t+m is masked for row r iff i >= thr[r, m]
    mk = consts.tile([P, 2, KCHUNK], BF16)
    for m in range(2):
        nc.vector.scalar_tensor_tensor(mk[:, m, :], iota_f,
                                       thr_sb[:, m:m + 1], negbig,
                                       op0=OP.is_ge, op1=OP.mult)

    # per-tile P^T (fp8, [kpos-block, q] layout) and chunk-sum slots
    ptjs = [pt_pool.tile([P, TILE_CHUNKS[t], 4, P], F8, tag=f"pt{t}",
                         name=f"ptj{t}")
            for t in range(N_QTILES)]
    ptb0 = consts.tile([P, P], BF16, name="ptb0")
    partials = [stat_pool.tile([P, SEQ_CHUNKS], F32, tag=f"pa{t}",
                               name=f"partials{t}")
                for t in range(N_QTILES)]

    p_exps = {}
    n_tcopy = 0
    o_tiles = {}

    def emit_s(t, j):
        """S chunk (fp8 DoubleRow) -> exp (scalar, accumulated sums).
        Non-boundary chunks exp straight from PSUM; boundary chunks fold
        the additive mask in a vector add first."""
        ktj = kts[j]
        sps = s_ps.tile([P, KCHUNK], F32, tag="s")
        for dp in range(D_TILES // 2):
            nc.tensor.matmul(sps,
                             qt_sb[:, 2 * dp:2 * dp + 2, t * P:(t + 1) * P],
                             ktj[:, 2 * dp:2 * dp + 2, :],
                             start=(dp == 0), stop=(dp == D_TILES // 2 - 1),
                             perf_mode=DR)
        p_sb = p_pool.tile([P, KCHUNK], BF16, tag="p")
        if j >= 2 * t:
            srt = stmp_pool.tile([P, KCHUNK], BF16, tag="stmp")
            nc.vector.tensor_tensor(srt, sps, mk[:, j - 2 * t, :], OP.add)
            nc.scalar.activation(p_sb, srt, ACT.Exp, scale=SM_SCALE,
                                 accum_out=partials[t][:, j:j + 1])
        else:
            nc.scalar.activation(p_sb, sps, ACT.Exp, scale=SM_SCALE,
                                 accum_out=partials[t][:, j:j + 1])
        p_exps[(t, j)] = p_sb

    def emit_transposes(t, j):
        nonlocal n_tcopy
        tps = t_ps.tile([P, KCHUNK], BF16)
        p_sb = p_exps.pop((t, j))
        for ks in range(4):
            nc.tensor.transpose(tps[:, ks * P:(ks + 1) * P],
                                p_sb[:, ks * P:(ks + 1) * P], ident)
        if t == 0 and j == 0:
            # kpos 0-127 keep a bf16 P^T (bf16 PV protects short rows)
            nc.vector.tensor_copy(ptb0, tps[:, 0:P])
            nc.scalar.copy(ptjs[0][:, 0, 1:4, :], tps[:, P:])
        elif n_tcopy % 2 == 0:
            nc.vector.tensor_copy(ptjs[t][:, j], tps)
        else:
            nc.scalar.copy(ptjs[t][:, j], tps)
        n_tcopy += 1

    def emit_pv(t, kcs, ops, stop):
        """PV chunks (fp8 DoubleRow; bf16 for tile 0's first kpos block)."""
        for kc in kcs:
            if t == 0 and kc == 0:
                for h in range(2):
                    nc.tensor.matmul(ops[:, h * KCHUNK:(h + 1) * KCHUNK],
                                     ptb0,
                                     vb0_sb[:, h * KCHUNK:(h + 1) * KCHUNK],
                                     start=True, stop=False)
                for h in range(2):
                    nc.tensor.matmul(ops[:, h * KCHUNK:(h + 1) * KCHUNK],
                                     ptjs[0][:, 0, 1:3, :],
                                     v_res[0][:, 1:3,
                                              h * KCHUNK:(h + 1) * KCHUNK],
                                     start=False, stop=False, perf_mode=DR)
                for h in range(2):
                    nc.tensor.matmul(ops[:, h * KCHUNK:(h + 1) * KCHUNK],
                                     ptjs[0][:, 0, 3, :],
                                     v_res[0][:, 3,
                                              h * KCHUNK:(h + 1) * KCHUNK],
                                     start=False, stop=False)
                continue
            for m in range(2):
                last = stop and kc == kcs[-1] and m == 1
                for h in range(2):
                    nc.tensor.matmul(ops[:, h * KCHUNK:(h + 1) * KCHUNK],
                                     ptjs[t][:, kc, 2 * m:2 * m + 2, :],
                                     v_res[kc][:, 2 * m:2 * m + 2,
                                               h * KCHUNK:(h + 1) * KCHUNK],
                                     start=(kc == 0 and m == 0),
                                     stop=last, perf_mode=DR)

    def emit_epilogue(t, ops):
        n = TILE_CHUNKS[t]
        rsum = stat_pool.tile([P, 1], F32, tag="stat")
        nc.vector.tensor_reduce(rsum, partials[t][:, :n], op=OP.add,
                                axis=AX.X)
        recip = stat_pool.tile([P, 1], F32, tag="stat")
        nc.vector.reciprocal(recip, rsum)
        # per-bank mul + DMA: the h0 half starts as soon as its PSUM bank
        # gets its stop-flag, overlapping the h1 half's last PV matmuls
        osb = osb_pool.tile([P, D], F32)
        for h in range(2):
            nc.vector.tensor_scalar_mul(osb[:, h * KCHUNK:(h + 1) * KCHUNK],
                                        ops[:, h * KCHUNK:(h + 1) * KCHUNK],
                                        recip)
            nc.gpsimd.dma_start(out=out_t[:, t, h * KCHUNK:(h + 1) * KCHUNK],
                                in_=osb[:, h * KCHUNK:(h + 1) * KCHUNK])

    # ---- fused stream ---------------------------------------------------
    for j in range(SEQ_CHUNKS + 1):
        # t descending: the deepest tile is on the critical path
        for t in reversed(range(N_QTILES)):
            if j < TILE_CHUNKS[t] and not (j == 7 and t == 3):
                emit_s(t, j)
        for t in reversed(range(N_QTILES)):
            if 1 <= j <= TILE_CHUNKS[t]:
                emit_transposes(t, j - 1)
        # PV as soon as a tile's P^T blocks are all (or nearly all) in:
        # short tiles complete right after their last transpose; tile 3
        # runs chunks 0-6 during round 7 (ahead of S(3,7), so a late kt7
        # can't stall the in-order tensor queue) and chunk 7 at round 8
        if j == 2 or j == 4 or j == 6:
            t = j // 2 - 1
            ops = o_ps.tile([P, D], F32)
            emit_pv(t, range(TILE_CHUNKS[t]), ops, stop=True)
            emit_epilogue(t, ops)
        elif j == 7:
            ops = o_ps.tile([P, D], F32)
            o_tiles[3] = ops
            emit_pv(3, range(7), ops, stop=False)
            emit_s(3, 7)
        elif j == 8:
            ops = o_tiles[3]
            emit_pv(3, [7], ops, stop=True)
            emit_epilogue(3, ops)


def _get_ncs():
    if "nc1" not in _CACHE:
        _CACHE["nc1"] = _build_nc1()
        _CACHE["nc2"] = _build_nc2()
    return _CACHE["nc1"], _CACHE["nc2"]


def _qcols(c):
    blocks = [8 * t + c for t in range(N_QTILES)]
    return blocks, np.concatenate(
        [np.arange(b * P, (b + 1) * P) for b in blocks])


def _perm_x(xT_slice):
    """[D, W] -> [128, 8, W] fp8 with di_inner on partitions."""
    W = xT_slice.shape[1]
    return np.ascontiguousarray(
        xT_slice.reshape(D_TILES, P, W).transpose(1, 0, 2)).astype(NP_F8)


def _perm_w_chunks(wT):
    """[d_in, d_out] -> [128, 8, 8, 128]: [di_p, do_o, di_o, do_i] fp8."""
    return np.ascontiguousarray(
        np.asarray(wT, np.float32).reshape(D_TILES, P, D_TILES, P)
        .transpose(1, 2, 0, 3)).astype(NP_F8)


def _perm_w_halves(wT):
    """[d_in, d_out] -> [128, 2, 8, 512]: [di_p, half, di_o, do_i] fp8."""
    return np.ascontiguousarray(
        np.asarray(wT, np.float32).reshape(D_TILES, P, 2, KCHUNK)
        .transpose(1, 2, 0, 3)).astype(NP_F8)


def _phase1_inmaps(xT, wqT, wkT, wvT):
    bf = ml_dtypes.bfloat16
    wk_p = _perm_w_chunks(wkT)
    wq_p = _perm_w_chunks(wqT)
    wv_p = _perm_w_halves(wvT)
    wvb_p = np.ascontiguousarray(
        np.asarray(wvT, np.float32).reshape(D_TILES, P, 2, KCHUNK)
        .transpose(1, 2, 0, 3)).astype(bf)
    xb_p = np.ascontiguousarray(
        np.asarray(xT[:, 0:P], np.float32)
        .reshape(D_TILES, P, P).transpose(1, 0, 2)).astype(bf)
    maps = []
    for c in range(N_CORES):
        _, cols = _qcols(c)
        maps.append({
            "xc": _perm_x(xT[:, c * KCHUNK:(c + 1) * KCHUNK]),
            "xq": _perm_x(xT[:, cols]),
            "wq": wq_p, "wk": wk_p, "wv": wv_p,
            "xb": xb_p, "wvb": wvb_p})
    return maps


def _gather_phase1(res1):
    ktf = np.stack([res1.results[c]["kt"] for c in range(N_CORES)])
    vf = np.stack([res1.results[c]["v"] for c in range(N_CORES)])
    qts = [res1.results[c]["qt"] for c in range(N_CORES)]
    vb0 = res1.results[0]["v0b"]
    return ktf, vf, qts, vb0


def _phase2_inmaps(ktf, vf, qts, vb0):
    maps = []
    r = np.arange(P, dtype=np.float32)
    for c in range(N_CORES):
        thr = np.empty((P, 2), np.float32)
        thr[:, 0] = 128 * c + r + 1
        thr[:, 1] = 128 * c + r + 1 - KCHUNK
        maps.append({"ktf": ktf, "vf": vf, "qt": qts[c], "vb0": vb0,
                     "thr": thr})
    return maps


def _run_spmd(nc, in_maps):
    """run_bass_kernel_spmd with retries: the first device touch after a
    crashed process occasionally reports NRT_EXEC_UNIT_UNRECOVERABLE once."""
    last = None
    for _ in range(3):
        try:
            return run_bass_kernel_spmd(nc, in_maps, list(range(N_CORES)))
        except Exception as e:  # transient device wedge
            last = e
    raise last


def kernel(x, w_q, w_k, w_v):
    nc1, nc2 = _get_ncs()
    x = np.asarray(x, np.float32)
    xT = np.ascontiguousarray(x.T)
    wqT = np.ascontiguousarray(np.asarray(w_q).T)
    wkT = np.ascontiguousarray(np.asarray(w_k).T)
    wvT = np.ascontiguousarray(np.asarray(w_v).T)

    res1 = _run_spmd(nc1, _phase1_inmaps(xT, wqT, wkT, wvT))
    res2 = _run_spmd(nc2, _phase2_inmaps(*_gather_phase1(res1)))

    full = np.empty((SEQ, D), np.float32)
    for c in range(N_CORES):
        oc = res2.results[c]["out"]
        blocks, _ = _qcols(c)
        for t, B in enumerate(blocks):
            full[B * P:(B + 1) * P, :] = oc[t * P:(t + 1) * P, :]
    return full


# revision 18
# speedup vs baseline: 1.2013x; 1.0350x over previous
"""Causal attention on 8 TRN2 NeuronCores — two-phase, fp8-heavy version.

Phase 1 (NEFF-1): Q/K/V projections, all in fp8 DoubleRow matmuls (x and
weights fp8-quantized on host). K/V sharded over seq across cores; Q^T for
the core's own (strided) row blocks. Outputs: K^T, Q^T, V in fp8 (V in
kpos-block layout for the fp8 PV matmuls of phase 2) plus a bf16 copy of
V's first 128 rows (v0b) — softmax rows with few terms need better-than-
fp8 V. Compute order K -> Q -> V so the tail output DMA is small.

Phase 2 (NEFF-2): flash-style causal attention, Q rows sharded over cores
(strided 128-row blocks), K^T / V streamed chunk-wise from DRAM in fp8.
Scores AND PV via fp8 DoubleRow matmuls (2x rate). Softmax without max
subtraction (logits are bounded): exp runs on the scalar engine straight
from PSUM with per-chunk accumulated sums; P^T via tensor-engine
transposes (cheap: they pipeline at ~61ns). The first 128 kpos of tile 0
multiply in bf16 (P^T bf16 x v0b) to protect short softmax rows. Causal
masking only touches the two boundary chunks per tile via two additive
masks whose thresholds are tile-independent. Each tile's PV + epilogue +
output DMA is interleaved into the chunk stream as soon as its P^T tiles
exist, so the kernel tail is just the last tile's boundary chunk.

DMA: the tiny thr tensor and the critical first inputs (qt block 3, kt0)
issue on the scalar queue ahead of the bulk stream; kt/v interleave on the
sync queue in consumption order; outputs go out on the gpsimd queue.
"""

import numpy as np
import ml_dtypes
from contextlib import ExitStack

import concourse.bass as bass
import concourse.tile as tile
from concourse import bacc, mybir
from concourse.bass_utils import run_bass_kernel_spmd
from concourse.masks import make_identity

P = 128
SEQ = 4096
D = 1024
N_CORES = 8
RPC = SEQ // N_CORES          # 512
D_TILES = D // P              # 8
KCHUNK = 512
SEQ_CHUNKS = SEQ // KCHUNK    # 8
N_QTILES = RPC // P           # 4
TILE_CHUNKS = [2, 4, 6, 8]
SM_SCALE = 1.0 / 32.0
NEG_BIG = -1.0e9

BF16 = mybir.dt.bfloat16
F32 = mybir.dt.float32
F8 = mybir.dt.float8e4
NP_F8 = ml_dtypes.float8_e4m3
DR = mybir.MatmulPerfMode.DoubleRow

_CACHE = {}


# ---------------------------------------------------------------- NEFF 1
def _build_nc1():
    nc = bacc.Bacc("TRN2", target_bir_lowering=False, debug=False,
                   num_devices=N_CORES)
    # partition-dim-first layouts, contiguous per partition
    xc = nc.dram_tensor("xc", [P, D_TILES, KCHUNK], F8,
                        kind="ExternalInput").ap()
    xq = nc.dram_tensor("xq", [P, D_TILES, RPC], F8,
                        kind="ExternalInput").ap()
    wk = nc.dram_tensor("wk", [P, D_TILES, D_TILES, P], F8,
                        kind="ExternalInput").ap()
    wq = nc.dram_tensor("wq", [P, D_TILES, D_TILES, P], F8,
                        kind="ExternalInput").ap()
    wv = nc.dram_tensor("wv", [P, 2, D_TILES, KCHUNK], F8,
                        kind="ExternalInput").ap()
    # bf16 copies of x rows 0-127 and w_v: V's first 128 rows must be
    # better than fp8 (short softmax rows see V noise unaveraged)
    xb = nc.dram_tensor("xb", [P, D_TILES, P], BF16,
                        kind="ExternalInput").ap()
    wvb = nc.dram_tensor("wvb", [P, 2, D_TILES, KCHUNK], BF16,
                         kind="ExternalInput").ap()
    kt_o = nc.dram_tensor("kt", [P, D_TILES, KCHUNK], F8,
                          kind="ExternalOutput").ap()
    qt_o = nc.dram_tensor("qt", [N_QTILES, P, D_TILES, P], F8,
                          kind="ExternalOutput").ap()
    v_o = nc.dram_tensor("v", [P, 4, D], F8, kind="ExternalOutput").ap()
    v0b_o = nc.dram_tensor("v0b", [P, D], BF16, kind="ExternalOutput").ap()

    with tile.TileContext(nc) as tc, ExitStack() as ctx:
        wpool = ctx.enter_context(tc.tile_pool(name="w", bufs=1))
        xpool = ctx.enter_context(tc.tile_pool(name="x", bufs=1))
        opool = ctx.enter_context(tc.tile_pool(name="o", bufs=1))
        ps = ctx.enter_context(tc.tile_pool(name="ps", bufs=6, space="PSUM"))

        # inputs stream on sync in consumption order (K -> Q -> V),
        # split so the first matmuls can start early
        xs = xpool.tile([P, D_TILES, KCHUNK], F8, tag="xs")
        nc.sync.dma_start(out=xs, in_=xc)
        wk_sb = wpool.tile([P, D_TILES, D_TILES, P], F8, tag="wk")
        nc.sync.dma_start(out=wk_sb[:, 0:1], in_=wk[:, 0:1])
        nc.sync.dma_start(out=wk_sb[:, 1:4], in_=wk[:, 1:4])
        nc.sync.dma_start(out=wk_sb[:, 4:8], in_=wk[:, 4:8])
        xq_sb = xpool.tile([P, D_TILES, RPC], F8, tag="xq")
        nc.sync.dma_start(out=xq_sb, in_=xq)
        wq_sb = wpool.tile([P, D_TILES, D_TILES, P], F8, tag="wq")
        nc.sync.dma_start(out=wq_sb[:, 0:4], in_=wq[:, 0:4])
        nc.sync.dma_start(out=wq_sb[:, 4:8], in_=wq[:, 4:8])
        wv_sb = wpool.tile([P, 2, D_TILES, KCHUNK], F8, tag="wv")
        nc.sync.dma_start(out=wv_sb, in_=wv)
        xb_sb = xpool.tile([P, D_TILES, P], BF16, tag="xb")
        nc.sync.dma_start(out=xb_sb, in_=xb)
        wvb_sb = wpool.tile([P, 2, D_TILES, KCHUNK], BF16, tag="wvb")
        nc.sync.dma_start(out=wvb_sb, in_=wvb)

        n_copy = 0

        def evac(dst, src):
            nonlocal n_copy
            if n_copy % 2 == 0:
                nc.vector.tensor_copy(dst, src)
            else:
                nc.scalar.copy(dst, src)
            n_copy += 1

        # K^T: [d_out, seq-chunk] fp8, one output DMA
        kt_sb = opool.tile([P, D_TILES, KCHUNK], F8, tag="kt")
        for do in range(D_TILES):
            p = ps.tile([P, KCHUNK], F32, tag="ps")
            for dp in range(D_TILES // 2):
                nc.tensor.matmul(p, wk_sb[:, do, 2 * dp:2 * dp + 2, :],
                                 xs[:, 2 * dp:2 * dp + 2, :],
                                 start=(dp == 0), stop=(dp == 3),
                                 perf_mode=DR)
            evac(kt_sb[:, do, :], p)
        nc.gpsimd.dma_start(out=kt_o, in_=kt_sb)

        # Q^T: fp8, block-major output (phase 2 fetches block 3 first)
        qt_sb = opool.tile([P, D_TILES, RPC], F8, tag="qt")
        for do in range(D_TILES):
            p = ps.tile([P, RPC], F32, tag="ps")
            for dp in range(D_TILES // 2):
                nc.tensor.matmul(p, wq_sb[:, do, 2 * dp:2 * dp + 2, :],
                                 xq_sb[:, 2 * dp:2 * dp + 2, :],
                                 start=(dp == 0), stop=(dp == 3),
                                 perf_mode=DR)
            evac(qt_sb[:, do, :], p)
        for b in range(N_QTILES):
            nc.gpsimd.dma_start(out=qt_o[b],
                                in_=qt_sb[:, :, b * P:(b + 1) * P])

        # V: [kpos-block, d_out] fp8 + bf16 copy of the first 128 rows
        v_sb = opool.tile([P, 4, D], F8, tag="v")
        v0b_sb = opool.tile([P, D], BF16, tag="v0b")
        for ks in range(4):
            for h in range(2):
                p = ps.tile([P, KCHUNK], F32, tag="ps")
                for dp in range(D_TILES // 2):
                    nc.tensor.matmul(p, xs[:, 2 * dp:2 * dp + 2,
                                           ks * P:(ks + 1) * P],
                                     wv_sb[:, h, 2 * dp:2 * dp + 2, :],
                                     start=(dp == 0), stop=(dp == 3),
                                     perf_mode=DR)
                evac(v_sb[:, ks, h * KCHUNK:(h + 1) * KCHUNK], p)
        nc.gpsimd.dma_start(out=v_o, in_=v_sb)
        # V rows 0-127 again, in bf16 from bf16 inputs
        for h in range(2):
            p = ps.tile([P, KCHUNK], F32, tag="ps")
            for di in range(D_TILES):
                nc.tensor.matmul(p, xb_sb[:, di, :], wvb_sb[:, h, di, :],
                                 start=(di == 0), stop=(di == D_TILES - 1))
            nc.scalar.copy(v0b_sb[:, h * KCHUNK:(h + 1) * KCHUNK], p)
        nc.gpsimd.dma_start(out=v0b_o, in_=v0b_sb)
    nc.compile()
    return nc


# ---------------------------------------------------------------- NEFF 2
def _build_nc2():
    nc = bacc.Bacc("TRN2", target_bir_lowering=False, debug=False,
                   num_devices=N_CORES)
    ktf = nc.dram_tensor("ktf", [SEQ_CHUNKS, P, D_TILES, KCHUNK], F8,
                         kind="ExternalInput").ap()
    vf = nc.dram_tensor("vf", [SEQ_CHUNKS, P, 4, D], F8,
                        kind="ExternalInput").ap()
    qt = nc.dram_tensor("qt", [N_QTILES, P, D_TILES, P], F8,
                        kind="ExternalInput").ap()
    vb0 = nc.dram_tensor("vb0", [P, D], BF16, kind="ExternalInput").ap()
    thr = nc.dram_tensor("thr", [P, 2], F32, kind="ExternalInput").ap()
    out = nc.dram_tensor("out", [RPC, D], F32, kind="ExternalOutput").ap()
    out_t = out.rearrange("(t p) f -> p t f", p=P)

    with tile.TileContext(nc) as tc, ExitStack() as ctx:
        _attention(ctx, tc, ktf, vf, qt, vb0, thr, out_t)
    nc.compile()
    return nc


def _attention(ctx, tc, ktf, vf, qt_in, vb0_in, thr_in, out_t):
    nc = tc.nc
    AX = mybir.AxisListType
    OP = mybir.AluOpType
    ACT = mybir.ActivationFunctionType

    consts = ctx.enter_context(tc.tile_pool(name="consts", bufs=1))
    qt_pool = ctx.enter_context(tc.tile_pool(name="qt", bufs=1))
    kt_pool = ctx.enter_context(tc.tile_pool(name="kt", bufs=1))
    vres_pool = ctx.enter_context(tc.tile_pool(name="vres", bufs=1))
    pt_pool = ctx.enter_context(tc.tile_pool(name="pt", bufs=1))
    p_pool = ctx.enter_context(tc.tile_pool(name="p", bufs=9))
    stmp_pool = ctx.enter_context(tc.tile_pool(name="stmp", bufs=4))
    osb_pool = ctx.enter_context(tc.tile_pool(name="osb", bufs=2))
    stat_pool = ctx.enter_context(tc.tile_pool(name="stat", bufs=16))

    s_ps = ctx.enter_context(tc.tile_pool(name="s_ps", bufs=2, space="PSUM"))
    t_ps = ctx.enter_context(tc.tile_pool(name="t_ps", bufs=2, space="PSUM"))
    o_ps = ctx.enter_context(tc.tile_pool(name="o_ps", bufs=2, space="PSUM"))

    # ALL inputs on one HWDGE queue in strict priority order — a second
    # queue's descriptors would round-robin into the same 16 rings and
    # halve the critical stream's bandwidth. thr (1 KB, gates the masks)
    # rides the otherwise-idle scalar queue so it lands instantly.
    thr_sb = consts.tile([P, 2], F32)
    nc.scalar.dma_start(out=thr_sb, in_=thr_in)
    qt_sb = qt_pool.tile([P, D_TILES, RPC], F8)
    kts = [kt_pool.tile([P, D_TILES, KCHUNK], F8, tag=f"kt{j}",
                        name=f"kt{j}")
           for j in range(SEQ_CHUNKS)]
    v_res = [vres_pool.tile([P, 4, D], F8, tag=f"v{j}", name=f"vres{j}")
             for j in range(SEQ_CHUNKS)]
    nc.sync.dma_start(out=qt_sb[:, :, 3 * P:4 * P], in_=qt_in[3])
    nc.sync.dma_start(out=kts[0][:, 0:4], in_=ktf[0][:, 0:4])
    nc.sync.dma_start(out=kts[0][:, 4:8], in_=ktf[0][:, 4:8])
    for b in (2, 1, 0):
        nc.sync.dma_start(out=qt_sb[:, :, b * P:(b + 1) * P], in_=qt_in[b])
    vb0_sb = consts.tile([P, D], BF16)
    # bulk stream in consumption order (S eats kt_j at round j, PV(t)
    # eats v_0..v_{2t+1} at round 2t+2)
    nc.sync.dma_start(out=kts[1], in_=ktf[1])
    nc.sync.dma_start(out=vb0_sb, in_=vb0_in)
    nc.sync.dma_start(out=v_res[0], in_=vf[0])
    nc.sync.dma_start(out=v_res[1], in_=vf[1])
    for j in range(2, SEQ_CHUNKS):
        nc.sync.dma_start(out=kts[j], in_=ktf[j])
        nc.sync.dma_start(out=v_res[j], in_=vf[j])

    ident = consts.tile([P, P], BF16)
    make_identity(nc, ident)
    iota_i = consts.tile([P, KCHUNK], mybir.dt.int32)
    nc.gpsimd.iota(iota_i, pattern=[[1, KCHUNK]], base=0, channel_multiplier=0)
    iota_f = consts.tile([P, KCHUNK], F32)
    nc.vector.tensor_copy(iota_f, iota_i)
    negbig = consts.tile([P, KCHUNK], F32)
    nc.gpsimd.memset(negbig, NEG_BIG)
    # additive causal masks for the two boundary chunks of every tile:
    # col i of chunk 2t+m is masked for row r iff i >= thr[r, m]
    mk = consts.tile([P, 2, KCHUNK], BF16)
    for m in range(2):
        nc.vector.scalar_tensor_tensor(mk[:, m, :], iota_f,
                                       thr_sb[:, m:m + 1], negbig,
                                       op0=OP.is_ge, op1=OP.mult)

    # per-tile P^T (fp8, [kpos-block, q] layout) and chunk-sum slots
    ptjs = [pt_pool.tile([P, TILE_CHUNKS[t], 4, P], F8, tag=f"pt{t}",
                         name=f"ptj{t}")
            for t in range(N_QTILES)]
    ptb0 = consts.tile([P, P], BF16, name="ptb0")
    partials = [stat_pool.tile([P, SEQ_CHUNKS], F32, tag=f"pa{t}",
                               name=f"partials{t}")
                for t in range(N_QTILES)]

    p_exps = {}
    n_tcopy = 0
    o_tiles = {}

    def emit_s(t, j):
        """S chunk (fp8 DoubleRow) -> exp (scalar, accumulated sums).
        Non-boundary chunks exp straight from PSUM; boundary chunks fold
        the additive mask in a vector add first."""
        ktj = kts[j]
        sps = s_ps.tile([P, KCHUNK], F32, tag="s")
        for dp in range(D_TILES // 2):
            nc.tensor.matmul(sps,
                             qt_sb[:, 2 * dp:2 * dp + 2, t * P:(t + 1) * P],
                             ktj[:, 2 * dp:2 * dp + 2, :],
                             start=(dp == 0), stop=(dp == D_TILES // 2 - 1),
                             perf_mode=DR)
        p_sb = p_pool.tile([P, KCHUNK], BF16, tag="p")
        if j >= 2 * t:
            srt = stmp_pool.tile([P, KCHUNK], BF16, tag="stmp")
            nc.vector.tensor_tensor(srt, sps, mk[:, j - 2 * t, :], OP.add)
            nc.scalar.activation(p_sb, srt, ACT.Exp, scale=SM_SCALE,
                                 accum_out=partials[t][:, j:j + 1])
        else:
            nc.scalar.activation(p_sb, sps, ACT.Exp, scale=SM_SCALE,
                                 accum_out=partials[t][:, j:j + 1])
        p_exps[(t, j)] = p_sb

    def emit_transposes(t, j):
        nonlocal n_tcopy
        tps = t_ps.tile([P, KCHUNK], BF16)
        p_sb = p_exps.pop((t, j))
        for ks in range(4):
            nc.tensor.transpose(tps[:, ks * P:(ks + 1) * P],
                                p_sb[:, ks * P:(ks + 1) * P], ident)
        if t == 0 and j == 0:
            # kpos 0-127 keep a bf16 P^T (bf16 PV protects short rows)
            nc.vector.tensor_copy(ptb0, tps[:, 0:P])
            nc.scalar.copy(ptjs[0][:, 0, 1:4, :], tps[:, P:])
        elif n_tcopy % 2 == 0:
            nc.vector.tensor_copy(ptjs[t][:, j], tps)
        else:
            nc.scalar.copy(ptjs[t][:, j], tps)
        n_tcopy += 1

    def emit_pv(t, kcs, ops, stop):
        """PV chunks (fp8 DoubleRow; bf16 for tile 0's first kpos block)."""
        for kc in kcs:
            if t == 0 and kc == 0:
                for h in range(2):
                    nc.tensor.matmul(ops[:, h * KCHUNK:(h + 1) * KCHUNK],
                                     ptb0,
                                     vb0_sb[:, h * KCHUNK:(h + 1) * KCHUNK],
                                     start=True, stop=False)
                for h in range(2):
                    nc.tensor.matmul(ops[:, h * KCHUNK:(h + 1) * KCHUNK],
                                     ptjs[0][:, 0, 1:3, :],
                                     v_res[0][:, 1:3,
                                              h * KCHUNK:(h + 1) * KCHUNK],
                                     start=False, stop=False, perf_mode=DR)
                for h in range(2):
                    nc.tensor.matmul(ops[:, h * KCHUNK:(h + 1) * KCHUNK],
                                     ptjs[0][:, 0, 3, :],
                                     v_res[0][:, 3,
                                              h * KCHUNK:(h + 1) * KCHUNK],
                                     start=False, stop=False)
                continue
            for m in range(2):
                last = stop and kc == kcs[-1] and m == 1
                for h in range(2):
                    nc.tensor.matmul(ops[:, h * KCHUNK:(h + 1) * KCHUNK],
                                     ptjs[t][:, kc, 2 * m:2 * m + 2, :],
                                     v_res[kc][:, 2 * m:2 * m + 2,
                                               h * KCHUNK:(h + 1) * KCHUNK],
                                     start=(kc == 0 and m == 0),
                                     stop=last, perf_mode=DR)

    def emit_epilogue(t, ops):
        n = TILE_CHUNKS[t]
        rsum = stat_pool.tile([P, 1], F32, tag="stat")
        nc.vector.tensor_reduce(rsum, partials[t][:, :n], op=OP.add,
                                axis=AX.X)
        recip = stat_pool.tile([P, 1], F32, tag="stat")
        nc.vector.reciprocal(recip, rsum)
        osb = osb_pool.tile([P, D], F32)
        nc.vector.tensor_scalar_mul(osb, ops, recip)
        nc.gpsimd.dma_start(out=out_t[:, t, :], in_=osb)

    # ---- fused stream ---------------------------------------------------
    for j in range(SEQ_CHUNKS + 1):
        # t descending: the deepest tile is on the critical path
        for t in reversed(range(N_QTILES)):
            if j < TILE_CHUNKS[t]:
                emit_s(t, j)
        for t in reversed(range(N_QTILES)):
            if 1 <= j <= TILE_CHUNKS[t]:
                emit_transposes(t, j - 1)
        # PV as soon as a tile's P^T blocks are all (or nearly all) in:
        # short tiles complete right after their last transpose; tile 3
        # runs chunks 0-6 during round 7 (ahead of S(3,7), so a late kt7
        # can't stall the in-order tensor queue) and chunk 7 at round 8
        if j == 2 or j == 4 or j == 6:
            t = j // 2 - 1
            ops = o_ps.tile([P, D], F32)
            emit_pv(t, range(TILE_CHUNKS[t]), ops, stop=True)
            emit_epilogue(t, ops)
        elif j == 7:
            ops = o_ps.tile([P, D], F32)
            o_tiles[3] = ops
            emit_pv(3, range(7), ops, stop=False)
        elif j == 8:
            ops = o_tiles[3]
            emit_pv(3, [7], ops, stop=True)
            emit_epilogue(3, ops)


def _get_ncs():
    if "nc1" not in _CACHE:
        _CACHE["nc1"] = _build_nc1()
        _CACHE["nc2"] = _build_nc2()
    return _CACHE["nc1"], _CACHE["nc2"]


def _qcols(c):
    blocks = [8 * t + c for t in range(N_QTILES)]
    return blocks, np.concatenate(
        [np.arange(b * P, (b + 1) * P) for b in blocks])


def _perm_x(xT_slice):
    """[D, W] -> [128, 8, W] fp8 with di_inner on partitions."""
    W = xT_slice.shape[1]
    return np.ascontiguousarray(
        xT_slice.reshape(D_TILES, P, W).transpose(1, 0, 2)).astype(NP_F8)


def _perm_w_chunks(wT):
    """[d_in, d_out] -> [128, 8, 8, 128]: [di_p, do_o, di_o, do_i] fp8."""
    return np.ascontiguousarray(
        np.asarray(wT, np.float32).reshape(D_TILES, P, D_TILES, P)
        .transpose(1, 2, 0, 3)).astype(NP_F8)


def _perm_w_halves(wT):
    """[d_in, d_out] -> [128, 2, 8, 512]: [di_p, half, di_o, do_i] fp8."""
    return np.ascontiguousarray(
        np.asarray(wT, np.float32).reshape(D_TILES, P, 2, KCHUNK)
        .transpose(1, 2, 0, 3)).astype(NP_F8)


def _phase1_inmaps(xT, wqT, wkT, wvT):
    bf = ml_dtypes.bfloat16
    wk_p = _perm_w_chunks(wkT)
    wq_p = _perm_w_chunks(wqT)
    wv_p = _perm_w_halves(wvT)
    wvb_p = np.ascontiguousarray(
        np.asarray(wvT, np.float32).reshape(D_TILES, P, 2, KCHUNK)
        .transpose(1, 2, 0, 3)).astype(bf)
    xb_p = np.ascontiguousarray(
        np.asarray(xT[:, 0:P], np.float32)
        .reshape(D_TILES, P, P).transpose(1, 0, 2)).astype(bf)
    maps = []
    for c in range(N_CORES):
        _, cols = _qcols(c)
        maps.append({
            "xc": _perm_x(xT[:, c * KCHUNK:(c + 1) * KCHUNK]),
            "xq": _perm_x(xT[:, cols]),
            "wq": wq_p, "wk": wk_p, "wv": wv_p,
            "xb": xb_p, "wvb": wvb_p})
    return maps


def _gather_phase1(res1):
    ktf = np.stack([res1.results[c]["kt"] for c in range(N_CORES)])
    vf = np.stack([res1.results[c]["v"] for c in range(N_CORES)])
    qts = [res1.results[c]["qt"] for c in range(N_CORES)]
    vb0 = res1.results[0]["v0b"]
    return ktf, vf, qts, vb0


def _phase2_inmaps(ktf, vf, qts, vb0):
    maps = []
    r = np.arange(P, dtype=np.float32)
    for c in range(N_CORES):
        thr = np.empty((P, 2), np.float32)
        thr[:, 0] = 128 * c + r + 1
        thr[:, 1] = 128 * c + r + 1 - KCHUNK
        maps.append({"ktf": ktf, "vf": vf, "qt": qts[c], "vb0": vb0,
                     "thr": thr})
    return maps


def _run_spmd(nc, in_maps):
    """run_bass_kernel_spmd with retries: the first device touch after a
    crashed process occasionally reports NRT_EXEC_UNIT_UNRECOVERABLE once."""
    last = None
    for _ in range(3):
        try:
            return run_bass_kernel_spmd(nc, in_maps, list(range(N_CORES)))
        except Exception as e:  # transient device wedge
            last = e
    raise last


def kernel(x, w_q, w_k, w_v):
    nc1, nc2 = _get_ncs()
    x = np.asarray(x, np.float32)
    xT = np.ascontiguousarray(x.T)
    wqT = np.ascontiguousarray(np.asarray(w_q).T)
    wkT = np.ascontiguousarray(np.asarray(w_k).T)
    wvT = np.ascontiguousarray(np.asarray(w_v).T)

    res1 = _run_spmd(nc1, _phase1_inmaps(xT, wqT, wkT, wvT))
    res2 = _run_spmd(nc2, _phase2_inmaps(*_gather_phase1(res1)))

    full = np.empty((SEQ, D), np.float32)
    for c in range(N_CORES):
        oc = res2.results[c]["out"]
        blocks, _ = _qcols(c)
        for t, B in enumerate(blocks):
            full[B * P:(B + 1) * P, :] = oc[t * P:(t + 1) * P, :]
    return full
